# revision 1
# baseline (speedup 1.0000x reference)
"""Asymmetric correlation kernel v5 — 4x32-position Gram tiles, M=128
matmuls, on-chip diagonal gather (no DRAM scratch), deskew via shifted
full-block PE transposes.

Per core (batch element): x1, x2 [256, 96, 160] f32 -> out [81, 96, 160].

Pipeline per tile t = (yb, xq), yb in 0..3, xq in 0..40:
  PE:   G[m, n] = sum_c x1[c, y, x] * x2[c, y+dh, x+dw] band Gram
        m = 4*yl + xj (yl in 32-row block, xj in 4-col block)
        n = 12*u + v (u in 40-row band, v in 12-col window), N=480
  DVE/Act: drain psum -> gram slot (bf16, offset 8)
  SP:   diagonal gather gram -> bandq2[m, (xq*3+yb)*114 + k]
        in [[ROWG+3, 128], [1, 112]]: run k holds G[m, 12yl+3xj-6+k]
  PE:   4 transposes per tile at offsets 6-2xj -> pt[k', m] with
        k' = 12di+dj for columns m = xj (mod 4)
  DVE/Act: evac pt -> asm2[k=12di+dj, y, x]
  Pool: 27 output DMAs (yb x di), bf16->f32 cast, partitions 12di+dj

Host: x1 scaled by 1/256 (exact) and packed [c, yb, xq, yl, xj] bf16;
x2 bf16; edge columns (x+dw out of range) zeroed in numpy.
"""

from contextlib import ExitStack

import numpy as np
import ml_dtypes

import concourse.bass as bass
import concourse.mybir as mybir
from concourse.bass_utils import run_bass_kernel_spmd

F32 = mybir.dt.float32
BF16 = mybir.dt.bfloat16

C = 256
H = 96
W = 160
ND = 81
YT = 32                  # y rows per tile
XQ = 4                   # x cols per tile
NYB = H // YT            # 3
NXQ = W // XQ            # 40
NT = NYB * NXQ           # 120 tiles
WU = YT + 8              # 40 band rows
WV = XQ + 8              # 12 band cols
N = WU * WV              # 480
HROW = (H + 8) * W       # 16640 per h-half of x2s
X2SZ = 4 + 2 * HROW + 12
RS = 496                 # gram slot size
NSL = 24                 # gram ring slots (4 write-batches)
ROWG = NSL * RS          # 8928
RUN = 112
TS = 480 * 128 + 6 + 122  # scratch tile pitch (6 front pad, tail slack)
BS = 6                   # tiles per scratch write/read batch
NRQ = 36                 # bandq ring slots
X1CH = NXQ * YT * XQ     # 5120 per (h, yb) chunk
ROWA = H * W             # asm2 row


def build(dbg=False):
    nc = bass.Bass("TRN2", target_bir_lowering=False, debug=False)

    x1 = nc.dram_tensor("x1", [C, NYB, X1CH], BF16, kind="ExternalInput")
    x2 = nc.dram_tensor("x2", [C, H, W], BF16, kind="ExternalInput")
    out = nc.dram_tensor("out", [ND, H, W], F32, kind="ExternalOutput")
    scratch = nc.dram_tensor("scratch", [NT, TS], BF16,
                             kind="ExternalOutput" if dbg else "Internal")
    if dbg:
        gramd = nc.dram_tensor("gramd", [128, ROWG], F32,
                               kind="ExternalOutput")
        bandqd = nc.dram_tensor("bandqd", [128, NRQ, 120], F32,
                                kind="ExternalOutput")
        asmd = nc.dram_tensor("asmd", [112, H, W], F32,
                              kind="ExternalOutput")
        x2sd = nc.dram_tensor("x2sd", [128, X2SZ], F32,
                              kind="ExternalOutput")

    with ExitStack() as ctx:
        ent = ctx.enter_context
        x1r = ent(nc.sbuf_tensor("x1r", [128, 2, 2, X1CH], BF16))
        x2s = ent(nc.sbuf_tensor("x2s", [128, X2SZ], BF16))
        gram = ent(nc.sbuf_tensor("gram", [128, ROWG], BF16))
        bandq = ent(nc.sbuf_tensor("bandq", [128, NRQ, 120], BF16))
        asm2 = ent(nc.sbuf_tensor("asm2", [112, H, W], F32))
        ident = ent(nc.sbuf_tensor("ident", [128, 128], BF16))

        pg = [ent(nc.psum_tensor(f"pg{i}", [128, N], F32)) for i in range(5)]
        pt = [ent(nc.psum_tensor(f"pt{i}", [112, 4, 128], BF16))
              for i in range(3)]

        s_init = ent(nc.semaphore("s_init"))    # ident+guards ready
        s_ldg = [ent(nc.semaphore(f"s_ldg{i}")) for i in range(3)]
        s_mm = ent(nc.semaphore("s_mm"))        # per tile
        s_drv = ent(nc.semaphore("s_drv"))      # drains (DVE, 1/tile)
        s_w = [ent(nc.semaphore(f"s_w{i}")) for i in range(4)]
        s_r = [ent(nc.semaphore(f"s_r{i}")) for i in range(6)]
        s_tp = ent(nc.semaphore("s_tp"))        # transposes (4/tile)
        s_ev = ent(nc.semaphore("s_ev"))        # evacs (Act, 1/tile)
        s_out = ent(nc.semaphore("s_out"))      # output DMAs (16/dma)

        # load DMA order (chunks of 32 rows, both h halves):
        # x2c0 x2c1 x1c0 | x2c2 x1c1 | x1c2  (h0,h1 pairs each)
        # tile row yb ready after: yb0: 6 dmas, yb1: 10, yb2: 12
        LD_THRESH = [96, 64, 64]  # per load-group dma counts x16

        def drain_evac(eng, copyf, t):
            """One tile's drain: psum pg[t%6] -> gram slot t%NSL."""
            eng.wait_ge(s_mm, t + 1)
            if t >= NSL:
                beta = (t - NSL) // BS
                eng.wait_ge(s_w[beta % 4], 16 * (beta // 4 + 1))
            sl = t % NSL
            copyf(
                bass.AP(tensor=gram, offset=sl * RS + 8,
                        ap=[[ROWG, 128], [1, N]]),
                pg[t % 5].ap(),
            ).then_inc(s_drv, 1)

        def evac(eng, copyf, t):
            yb, xq = t // NXQ, t % NXQ
            eng.wait_ge(s_tp, 4 * (t + 1))
            copyf(
                bass.AP(tensor=asm2, offset=YT * yb * W + XQ * xq,
                        ap=[[ROWA, 112], [1, 4], [W, YT]]),
                bass.AP(tensor=pt[t % 3], offset=0,
                        ap=[[4 * 128, 112], [129, 4], [4, YT]]),
            ).then_inc(s_ev, 1)

        with nc.Block() as block:

            @block.gpsimd
            def _(gp):
                gp.wait_ge(s_init, 1)
                gp.affine_select(
                    out=ident.ap(), in_=ident.ap(),
                    compare_op=mybir.AluOpType.not_equal,
                    fill=1.0, base=0, pattern=[[-1, 128]],
                    channel_multiplier=1,
                ).then_inc(s_init, 1)
                if dbg:
                    gp.wait_ge(s_ev, NT)
                    gp.dma_start(out=gramd.ap(), in_=gram.ap()
                                 ).then_inc(s_out, 16)
                    gp.dma_start(out=bandqd.ap(), in_=bandq.ap()
                                 ).then_inc(s_out, 16)
                    gp.dma_start(out=asmd.ap(), in_=asm2.ap()
                                 ).then_inc(s_out, 16)
                    gp.dma_start(out=x2sd.ap(), in_=x2s.ap()
                                 ).then_inc(s_out, 16)

            @block.vector
            def _(vec):
                vec.memset(ident.ap(), 0.0).then_inc(s_init, 1)
                # x2s zero guards: [0,644) [16004,17284) [32644,33296)
                vec.memset(x2s.ap()[:, 0:4 + 4 * W], 0.0)
                vec.memset(
                    x2s.ap()[:, 4 + 100 * W: 4 + HROW + 4 * W], 0.0)
                vec.memset(x2s.ap()[:, 4 + HROW + 100 * W:], 0.0
                           ).then_inc(s_init, 1)
                for t in range(NT):
                    drain_evac(vec, vec.tensor_copy, t)

            @block.scalar
            def _(act):
                def out_dma(yb, di):
                    act.wait_ge(s_ev, NXQ * (yb + 1))
                    in_ = bass.AP(
                        tensor=asm2,
                        offset=12 * di * ROWA + YT * yb * W,
                        ap=[[ROWA, 9], [1, YT * W]])
                    o = bass.AP(
                        tensor=out,
                        offset=9 * di * H * W + YT * yb * W,
                        ap=[[H * W, 9], [1, YT * W]])
                    act.dma_start(out=o, in_=in_).then_inc(s_out, 16)

                # spread each row's 9 output issues over later evac slots
                pend = []
                for t in range(NT):
                    evac(act, act.copy, t)
                    if t % NXQ == NXQ - 1 and t < NT - 1:
                        yb = t // NXQ
                        pend.extend((yb, di) for di in range(9))
                    if pend and t % 4 == 3:
                        out_dma(*pend.pop(0))
                for yb, di in pend:
                    out_dma(yb, di)
                for di in range(9):
                    out_dma(2, di)

            @block.sync
            def _(sp):
                def write_batch(b):
                    t0 = BS * b
                    sp.wait_ge(s_drv, t0 + BS)
                    sl0 = t0 % NSL
                    in_ap = bass.AP(tensor=gram, offset=sl0 * RS + 8,
                                    ap=[[ROWG, 128], [RS, BS], [1, N]])
                    out_ap = bass.AP(tensor=scratch, offset=t0 * TS + 6,
                                     ap=[[480, 128], [TS, BS], [1, N]])
                    sp.dma_start(out=out_ap, in_=in_ap
                                 ).then_inc(s_w[b % 4], 16)

                def read_batch(b):
                    t0 = BS * b
                    sp.wait_ge(s_w[b % 4], 16 * (b // 4 + 1))
                    if b >= NRQ // BS:
                        done = BS * (b - NRQ // BS) + BS
                        sp.wait_ge(s_tp, 4 * done)
                    in_ap = bass.AP(tensor=scratch, offset=t0 * TS,
                                    ap=[[483, 128], [TS, BS], [1, 118]])
                    out_ap = bass.AP(tensor=bandq,
                                     offset=(t0 % NRQ) * 120,
                                     ap=[[NRQ * 120, 128], [120, BS],
                                         [1, 118]])
                    sp.dma_start(out=out_ap, in_=in_ap
                                 ).then_inc(s_r[b % 6], 16)

                def ld_x1(g, h):
                    in_ = bass.AP(tensor=x1, offset=128 * h * NYB * X1CH
                                  + g * X1CH,
                                  ap=[[NYB * X1CH, 128], [1, X1CH]])
                    sp.dma_start(
                        out=x1r.ap()[:, h, g % 2, :], in_=in_
                        ).then_inc(s_ldg[ldg[0]], 16)

                def ld_x2r(r0, r1, h):
                    in_ = bass.AP(tensor=x2, offset=128 * h * H * W + r0 * W,
                                  ap=[[H * W, 128], [1, (r1 - r0) * W]])
                    o = 4 + h * HROW + (r0 + 4) * W
                    sp.dma_start(
                        out=bass.AP(tensor=x2s, offset=o,
                                    ap=[[X2SZ, 128], [1, (r1 - r0) * W]]),
                        in_=in_).then_inc(s_ldg[ldg[0]], 16)

                ldg = [0]
                for h in range(2):
                    ld_x2r(0, 32, h)
                for h in range(2):
                    ld_x2r(32, 36, h)
                for h in range(2):
                    ld_x1(0, h)
                ldg[0] = 1
                for h in range(2):
                    ld_x2r(36, 64, h)
                for h in range(2):
                    ld_x1(1, h)
                ldg[0] = 2
                for h in range(2):
                    ld_x2r(64, 96, h)

                NB = NT // BS
                write_batch(0)
                for b in range(1, NB):
                    write_batch(b)
                    read_batch(b - 1)
                    if b == 6:
                        for h in range(2):
                            ld_x1(2, h)
                read_batch(NB - 1)

            @block.tensor
            def _(pe):
                pe.wait_ge(s_init, 3)
                LAG = 36

                def transposes(t):
                    b = t // BS
                    pe.wait_ge(s_r[b % 6], 16 * (b // 6 + 1))
                    if t >= 3:
                        pe.wait_ge(s_ev, t - 2)
                    base = (t % NRQ) * 120
                    for xj in range(XQ):
                        in_ap = bass.AP(tensor=bandq,
                                        offset=base + 6 - 2 * xj,
                                        ap=[[NRQ * 120, 128], [1, RUN]])
                        nc.tensor.transpose(
                            pt[t % 3].ap()[:, xj, :], in_ap, ident.ap()
                        ).then_inc(s_tp, 1)

                ntp = 0  # transposes emitted (by tile)
                for t in range(NT):
                    yb, xq = t // NXQ, t % NXQ
                    if xq == 0:
                        for g in range(yb + 1):
                            pe.wait_ge(s_ldg[g], LD_THRESH[g])
                    if t >= 5:
                        pe.wait_ge(s_drv, t - 4)
                    for h in range(2):
                        lhsT = bass.AP(
                            tensor=x1r,
                            offset=h * 2 * X1CH + (yb % 2) * X1CH + xq * 128,
                            ap=[[2 * 2 * X1CH, 128], [1, 128]])
                        rhs = bass.AP(
                            tensor=x2s,
                            offset=h * HROW + YT * yb * W + XQ * xq,
                            ap=[[X2SZ, 128], [W, WU], [1, WV]])
                        mm = nc.tensor.matmul(
                            pg[t % 5].ap(), lhsT, rhs,
                            start=(h == 0), stop=(h == 1))
                    mm.then_inc(s_mm, 1)
                    want = t - LAG + 1          # steady-state target
                    if t >= NT - 20:            # tail: catch up 2/iter
                        want = ntp + 2
                    while ntp < min(max(want, 0), NT):
                        transposes(ntp)
                        ntp += 1
                while ntp < NT:
                    transposes(ntp)
                    ntp += 1

    return nc


def kernel(x1, x2, trace=False):
    n = x1.shape[0]
    nc = build()
    bf = ml_dtypes.bfloat16
    in_maps = []
    for i in range(n):
        x1b = (x1[i].astype(np.float32) * (1.0 / C)).astype(bf)
        x1t = np.ascontiguousarray(
            x1b.reshape(C, NYB, YT, NXQ, XQ).transpose(0, 1, 3, 2, 4)
        ).reshape(C, NYB, X1CH)
        x2b = np.ascontiguousarray(x2[i]).astype(bf)
        in_maps.append({"x1": x1t, "x2": x2b})
    res = run_bass_kernel_spmd(nc, in_maps, list(range(n)), trace=trace)
    outv = np.stack([r["out"] for r in res.results], axis=0)
    # zero out-of-range x+dw edge columns (host-side fixup)
    for dj in range(9):
        dw = dj - 4
        if dw < 0:
            outv[:, dj::9, :, 0:-dw] = 0.0
        elif dw > 0:
            outv[:, dj::9, :, W - dw:] = 0.0
    if trace:
        kernel.last_exec_time_ns = res.exec_time_ns
        kernel.last_trace = res.instructions_and_trace
    return outv



# revision 11
# speedup vs baseline: 1.0237x; 1.0237x over previous
"""Asymmetric correlation kernel v5 — 4x32-position Gram tiles, M=128
matmuls, on-chip diagonal gather (no DRAM scratch), deskew via shifted
full-block PE transposes.

Per core (batch element): x1, x2 [256, 96, 160] f32 -> out [81, 96, 160].

Pipeline per tile t = (yb, xq), yb in 0..3, xq in 0..40:
  PE:   G[m, n] = sum_c x1[c, y, x] * x2[c, y+dh, x+dw] band Gram
        m = 4*yl + xj (yl in 32-row block, xj in 4-col block)
        n = 12*u + v (u in 40-row band, v in 12-col window), N=480
  DVE/Act: drain psum -> gram slot (bf16, offset 8)
  SP:   diagonal gather gram -> bandq2[m, (xq*3+yb)*114 + k]
        in [[ROWG+3, 128], [1, 112]]: run k holds G[m, 12yl+3xj-6+k]
  PE:   4 transposes per tile at offsets 6-2xj -> pt[k', m] with
        k' = 12di+dj for columns m = xj (mod 4)
  DVE/Act: evac pt -> asm2[k=12di+dj, y, x]
  Pool: 27 output DMAs (yb x di), bf16->f32 cast, partitions 12di+dj

Host: x1 scaled by 1/256 (exact) and packed [c, yb, xq, yl, xj] bf16;
x2 bf16; edge columns (x+dw out of range) zeroed in numpy.
"""

from contextlib import ExitStack

import numpy as np
import ml_dtypes

import concourse.bass as bass
import concourse.mybir as mybir
from concourse.bass_utils import run_bass_kernel_spmd

F32 = mybir.dt.float32
BF16 = mybir.dt.bfloat16

C = 256
H = 96
W = 160
ND = 81
YT = 32                  # y rows per tile
XQ = 4                   # x cols per tile
NYB = H // YT            # 3
NXQ = W // XQ            # 40
NT = NYB * NXQ           # 120 tiles
WU = YT + 8              # 40 band rows
WV = XQ + 8              # 12 band cols
N = WU * WV              # 480
HROW = (H + 8) * W       # 16640 per h-half of x2s
X2SZ = 4 + 2 * HROW + 12
RS = 496                 # gram slot size
NSL = 24                 # gram ring slots (4 write-batches)
ROWG = NSL * RS          # 8928
RUN = 112
# split scratch layout: per tile, half g holds partitions 64g..64g+63 with
# only the column window its diagonal band needs (n in [-6,307) / [186,480))
W0 = 313                 # half-0 window width (n = -6..306)
W1 = 294                 # half-1 window width (n = 186..479)
B1 = 64 * W0             # 20032, half-1 block base within tile
TS = B1 + 64 * W1 + 64   # scratch tile pitch (tail slack for read overrun)
BS = 6                   # tiles per scratch write/read batch
NRQ = 36                 # bandq ring slots
X1CH = NXQ * YT * XQ     # 5120 per (h, yb) chunk
ROWA = H * W             # asm2 row


def build(dbg=False):
    nc = bass.Bass("TRN2", target_bir_lowering=False, debug=False)

    x1 = nc.dram_tensor("x1", [C, NYB, X1CH], BF16, kind="ExternalInput")
    x2 = nc.dram_tensor("x2", [C, H, W], BF16, kind="ExternalInput")
    out = nc.dram_tensor("out", [ND, H, W], BF16, kind="ExternalOutput")
    scratch = nc.dram_tensor("scratch", [NT, TS], BF16,
                             kind="ExternalOutput" if dbg else "Internal")
    if dbg:
        gramd = nc.dram_tensor("gramd", [128, ROWG], F32,
                               kind="ExternalOutput")
        bandqd = nc.dram_tensor("bandqd", [128, NRQ, 120], F32,
                                kind="ExternalOutput")
        asmd = nc.dram_tensor("asmd", [112, H, W], F32,
                              kind="ExternalOutput")
        x2sd = nc.dram_tensor("x2sd", [128, X2SZ], F32,
                              kind="ExternalOutput")

    with ExitStack() as ctx:
        ent = ctx.enter_context
        x1r = ent(nc.sbuf_tensor("x1r", [128, 2, 2, X1CH], BF16))
        x2s = ent(nc.sbuf_tensor("x2s", [128, X2SZ], BF16))
        gram = ent(nc.sbuf_tensor("gram", [128, ROWG], BF16))
        bandq = ent(nc.sbuf_tensor("bandq", [128, NRQ, 120], BF16))
        asm2 = ent(nc.sbuf_tensor("asm2", [112, H, W], BF16))
        ident = ent(nc.sbuf_tensor("ident", [128, 128], BF16))

        pg = [ent(nc.psum_tensor(f"pg{i}", [128, N], F32)) for i in range(5)]
        pt = [ent(nc.psum_tensor(f"pt{i}", [112, 4, 128], BF16))
              for i in range(3)]

        s_init = ent(nc.semaphore("s_init"))    # ident+guards ready
        s_ldg = [ent(nc.semaphore(f"s_ldg{i}")) for i in range(3)]
        s_mm = ent(nc.semaphore("s_mm"))        # per tile
        s_drv = ent(nc.semaphore("s_drv"))      # drains (DVE, 1/tile)
        s_w = [ent(nc.semaphore(f"s_w{i}")) for i in range(4)]
        s_r = [ent(nc.semaphore(f"s_r{i}")) for i in range(6)]
        s_tp = ent(nc.semaphore("s_tp"))        # transposes (4/tile)
        s_ev = ent(nc.semaphore("s_ev"))        # evacs (Act, 1/tile)
        s_out = ent(nc.semaphore("s_out"))      # output DMAs (16/dma)

        # load DMA order (chunks of 32 rows, both h halves):
        # x2c0 x2c1 x1c0 | x2c2 x1c1 | x1c2  (h0,h1 pairs each)
        # tile row yb ready after: yb0: 6 dmas, yb1: 10, yb2: 12
        LD_THRESH = [96, 64, 64]  # per load-group dma counts x16

        def drain_evac(eng, copyf, t):
            """One tile's drain: psum pg[t%6] -> gram slot t%NSL."""
            eng.wait_ge(s_mm, t + 1)
            if t >= NSL:
                beta = (t - NSL) // BS
                eng.wait_ge(s_w[beta % 4], 32 * (beta // 4 + 1))
            sl = t % NSL
            copyf(
                bass.AP(tensor=gram, offset=sl * RS + 8,
                        ap=[[ROWG, 128], [1, N]]),
                pg[t % 5].ap(),
            ).then_inc(s_drv, 1)

        def evac(eng, copyf, t):
            yb, xq = t // NXQ, t % NXQ
            eng.wait_ge(s_tp, 4 * (t + 1))
            copyf(
                bass.AP(tensor=asm2, offset=YT * yb * W + XQ * xq,
                        ap=[[ROWA, 112], [1, 4], [W, YT]]),
                bass.AP(tensor=pt[t % 3], offset=0,
                        ap=[[4 * 128, 112], [129, 4], [4, YT]]),
            ).then_inc(s_ev, 1)

        with nc.Block() as block:

            @block.gpsimd
            def _(gp):
                gp.wait_ge(s_init, 1)
                gp.affine_select(
                    out=ident.ap(), in_=ident.ap(),
                    compare_op=mybir.AluOpType.not_equal,
                    fill=1.0, base=0, pattern=[[-1, 128]],
                    channel_multiplier=1,
                ).then_inc(s_init, 1)

                def read_batch(b):
                    t0 = BS * b
                    gp.wait_ge(s_w[b % 4], 32 * (b // 4 + 1))
                    if b >= NRQ // BS:
                        done = BS * (b - NRQ // BS) + BS
                        gp.wait_ge(s_tp, 4 * done)
                    # half 0: row p at W0*p, diag start col 3p-(-6)... flat
                    # addr = W0*p + 3p = 316p; k=0 <-> n = 3p-6
                    gp.dma_start(
                        out=bass.AP(tensor=bandq, offset=(t0 % NRQ) * 120,
                                    ap=[[NRQ * 120, 64], [120, BS], [1, 118]]),
                        in_=bass.AP(tensor=scratch, offset=t0 * TS,
                                    ap=[[W0 + 3, 64], [TS, BS], [1, 118]]),
                    ).then_inc(s_r[b % 6], 16)
                    # half 1: addr = B1 + (p-64)*W1 + 3p - 192
                    gp.dma_start(
                        out=bass.AP(tensor=bandq,
                                    offset=64 * NRQ * 120 + (t0 % NRQ) * 120,
                                    ap=[[NRQ * 120, 64], [120, BS], [1, 118]]),
                        in_=bass.AP(tensor=scratch, offset=t0 * TS + B1,
                                    ap=[[W1 + 3, 64], [TS, BS], [1, 118]]),
                    ).then_inc(s_r[b % 6], 16)

                NB = NT // BS
                for b in range(NB):
                    read_batch(b)
                if dbg:
                    gp.wait_ge(s_ev, NT)
                    gp.dma_start(out=gramd.ap(), in_=gram.ap()
                                 ).then_inc(s_out, 16)
                    gp.dma_start(out=bandqd.ap(), in_=bandq.ap()
                                 ).then_inc(s_out, 16)
                    gp.dma_start(out=asmd.ap(), in_=asm2.ap()
                                 ).then_inc(s_out, 16)
                    gp.dma_start(out=x2sd.ap(), in_=x2s.ap()
                                 ).then_inc(s_out, 16)

            @block.vector
            def _(vec):
                vec.memset(ident.ap(), 0.0).then_inc(s_init, 1)
                # x2s zero guards: [0,644) [16004,17284) [32644,33296)
                vec.memset(x2s.ap()[:, 0:4 + 4 * W], 0.0)
                vec.memset(
                    x2s.ap()[:, 4 + 100 * W: 4 + HROW + 4 * W], 0.0)
                vec.memset(x2s.ap()[:, 4 + HROW + 100 * W:], 0.0
                           ).then_inc(s_init, 1)
                for t in range(NT):
                    drain_evac(vec, vec.tensor_copy, t)

            @block.scalar
            def _(act):
                def out_dma(yb, di):
                    act.wait_ge(s_ev, NXQ * (yb + 1))
                    in_ = bass.AP(
                        tensor=asm2,
                        offset=12 * di * ROWA + YT * yb * W,
                        ap=[[ROWA, 9], [1, YT * W]])
                    o = bass.AP(
                        tensor=out,
                        offset=9 * di * H * W + YT * yb * W,
                        ap=[[H * W, 9], [1, YT * W]])
                    act.dma_start(out=o, in_=in_).then_inc(s_out, 16)

                # spread each row's 9 output issues over later evac slots
                pend = []
                for t in range(NT):
                    evac(act, act.copy, t)
                    if t % NXQ == NXQ - 1 and t < NT - 1:
                        yb = t // NXQ
                        pend.extend((yb, di) for di in range(9))
                    if pend and t % 4 == 3:
                        out_dma(*pend.pop(0))
                for yb, di in pend:
                    out_dma(yb, di)
                for di in range(9):
                    out_dma(2, di)

            @block.sync
            def _(sp):
                def write_batch(b):
                    t0 = BS * b
                    sp.wait_ge(s_drv, t0 + BS)
                    sl0 = t0 % NSL
                    # half 0: partitions 0..63, gram cols [2, 315) = n [-6, 307)
                    sp.dma_start(
                        out=bass.AP(tensor=scratch, offset=t0 * TS,
                                    ap=[[W0, 64], [TS, BS], [1, W0]]),
                        in_=bass.AP(tensor=gram, offset=sl0 * RS + 2,
                                    ap=[[ROWG, 64], [RS, BS], [1, W0]]),
                    ).then_inc(s_w[b % 4], 16)
                    # half 1: partitions 64..127, gram cols [194, 488)
                    sp.dma_start(
                        out=bass.AP(tensor=scratch, offset=t0 * TS + B1,
                                    ap=[[W1, 64], [TS, BS], [1, W1]]),
                        in_=bass.AP(tensor=gram,
                                    offset=64 * ROWG + sl0 * RS + 194,
                                    ap=[[ROWG, 64], [RS, BS], [1, W1]]),
                    ).then_inc(s_w[b % 4], 16)

                def ld_x1(g, h):
                    in_ = bass.AP(tensor=x1, offset=128 * h * NYB * X1CH
                                  + g * X1CH,
                                  ap=[[NYB * X1CH, 128], [1, X1CH]])
                    sp.dma_start(
                        out=x1r.ap()[:, h, g % 2, :], in_=in_
                        ).then_inc(s_ldg[ldg[0]], 16)

                def ld_x2r(r0, r1, h):
                    in_ = bass.AP(tensor=x2, offset=128 * h * H * W + r0 * W,
                                  ap=[[H * W, 128], [1, (r1 - r0) * W]])
                    o = 4 + h * HROW + (r0 + 4) * W
                    sp.dma_start(
                        out=bass.AP(tensor=x2s, offset=o,
                                    ap=[[X2SZ, 128], [1, (r1 - r0) * W]]),
                        in_=in_).then_inc(s_ldg[ldg[0]], 16)

                ldg = [0]
                for h in range(2):
                    ld_x2r(0, 32, h)
                for h in range(2):
                    ld_x2r(32, 36, h)
                for h in range(2):
                    ld_x1(0, h)
                ldg[0] = 1
                for h in range(2):
                    ld_x2r(36, 64, h)
                for h in range(2):
                    ld_x1(1, h)
                ldg[0] = 2
                for h in range(2):
                    ld_x2r(64, 96, h)

                NB = NT // BS
                write_batch(0)
                for b in range(1, NB):
                    write_batch(b)
                    if b == 6:
                        for h in range(2):
                            ld_x1(2, h)

            @block.tensor
            def _(pe):
                pe.wait_ge(s_init, 3)
                LAG = 36

                def transposes(t):
                    b = t // BS
                    pe.wait_ge(s_r[b % 6], 32 * (b // 6 + 1))
                    if t >= 3:
                        pe.wait_ge(s_ev, t - 2)
                    base = (t % NRQ) * 120
                    for xj in range(XQ):
                        in_ap = bass.AP(tensor=bandq,
                                        offset=base + 6 - 2 * xj,
                                        ap=[[NRQ * 120, 128], [1, RUN]])
                        nc.tensor.transpose(
                            pt[t % 3].ap()[:, xj, :], in_ap, ident.ap()
                        ).then_inc(s_tp, 1)

                ntp = 0  # transposes emitted (by tile)
                for t in range(NT):
                    yb, xq = t // NXQ, t % NXQ
                    if xq == 0:
                        for g in range(yb + 1):
                            pe.wait_ge(s_ldg[g], LD_THRESH[g])
                    if t >= 5:
                        pe.wait_ge(s_drv, t - 4)
                    for h in range(2):
                        lhsT = bass.AP(
                            tensor=x1r,
                            offset=h * 2 * X1CH + (yb % 2) * X1CH + xq * 128,
                            ap=[[2 * 2 * X1CH, 128], [1, 128]])
                        rhs = bass.AP(
                            tensor=x2s,
                            offset=h * HROW + YT * yb * W + XQ * xq,
                            ap=[[X2SZ, 128], [W, WU], [1, WV]])
                        mm = nc.tensor.matmul(
                            pg[t % 5].ap(), lhsT, rhs,
                            start=(h == 0), stop=(h == 1))
                    mm.then_inc(s_mm, 1)
                    want = t - LAG + 1          # steady-state target
                    if t >= NT - 20:            # tail: catch up 2/iter
                        want = ntp + 2
                    while ntp < min(max(want, 0), NT):
                        transposes(ntp)
                        ntp += 1
                while ntp < NT:
                    transposes(ntp)
                    ntp += 1

    return nc


def kernel(x1, x2, trace=False):
    n = x1.shape[0]
    nc = build()
    bf = ml_dtypes.bfloat16
    in_maps = []
    for i in range(n):
        x1b = (x1[i].astype(np.float32) * (1.0 / C)).astype(bf)
        x1t = np.ascontiguousarray(
            x1b.reshape(C, NYB, YT, NXQ, XQ).transpose(0, 1, 3, 2, 4)
        ).reshape(C, NYB, X1CH)
        x2b = np.ascontiguousarray(x2[i]).astype(bf)
        in_maps.append({"x1": x1t, "x2": x2b})
    res = run_bass_kernel_spmd(nc, in_maps, list(range(n)), trace=trace)
    outv = np.stack([r["out"] for r in res.results], axis=0).astype(np.float32)
    # zero out-of-range x+dw edge columns (host-side fixup)
    for dj in range(9):
        dw = dj - 4
        if dw < 0:
            outv[:, dj::9, :, 0:-dw] = 0.0
        elif dw > 0:
            outv[:, dj::9, :, W - dw:] = 0.0
    if trace:
        kernel.last_exec_time_ns = res.exec_time_ns
        kernel.last_trace = res.instructions_and_trace
    return outv

